# revision 47
# baseline (speedup 1.0000x reference)
"""TRN2 Bass kernel for nn_DiffQuantumSimulator (QAOA MaxCut, 18 qubits, p=4).

Strategy: data-parallel over batch (8 graphs -> 8 NeuronCores). Per core the
2^18 statevector lives in SBUF as [128 partitions x 2048] (re/im fp16 split).

Each QAOA layer applies exp(-i*hp) (diagonal) and the mixer RX(beta)^(x)18.
The mixer runs in 3 TensorE matmul phases:
  A: 128x128 complex gate RX^(x)7 on the 7 partition bits, fused with a
     partition<->free-bit transpose by using the *state* as the stationary
     operand (out = state_tile^T @ [C|D]).
  B: same trick on the next 7 bits.
  C: standard matmul applying RX^(x)4 (x) I_8 to the remaining 4 bits.
All matmuls run in float16 (full PE rate, half-cost LDWEIGHTS vs fp32r).

The A output (t1) is drained re/im-split for phase B's single-stride
stationary windows (stationary APs allow only one free dim); the B output
(t2) keeps PSUM's interleaved layout (per 128-col window: re|im) so each
B drain is one contiguous copy, and phase C reads it through a strided
moving AP. Phase C uses one PSUM bank per 512-chunk so the rotation's
PSUM reads never serialize other chunks' matmuls.

The inter-layer rotation (multiply by exp(-i*hp)) reads phase C's PSUM
directly via vector-engine scalar_tensor_tensor ops (plain fp16
tensor_tensor is ~2x slower on DVE than STT); gpsimd handles SBUF-only
combine ops, and the boundary chunk takes an all-DVE low-latency path.
Device returns 8 per-partition energy partial columns; host reduces and
scales.
"""

import numpy as np

import concourse.bass as bass
import concourse.mybir as mybir
import concourse.tile as tile
from concourse import bacc
from concourse.bass_utils import run_bass_kernel_spmd

N = 18
DIM = 1 << N
P = 128
F = DIM // P  # 2048
LAYERS = 4
BATCH = 8
NCORES = 8

FP32 = mybir.dt.float32
FP16 = mybir.dt.float16
ALU = mybir.AluOpType
ACT = mybir.ActivationFunctionType

# ----------------------------------------------------------------------------
# Host-side math: hp diagonal, gate matrices, bit-layout permutations
# ----------------------------------------------------------------------------


def _compute_hp(adj):
    W = (np.triu(adj, k=1) > 0.5).astype(np.float64)
    n_edges = W.sum()
    idx = np.arange(DIM)
    shifts = (N - 1 - np.arange(N))[:, None]
    Z = 1.0 - 2.0 * ((idx[None, :] >> shifts) & 1).astype(np.float64)
    T = W @ Z
    cross = np.einsum("ud,ud->d", T, Z)
    return 0.5 * (n_edges - cross)  # [DIM], integer-valued*0.5, exact


def _rx(beta):
    c, s = np.cos(beta), np.sin(beta)
    return np.array([[c, -1j * s], [-1j * s, c]], dtype=np.complex128)


def _kron_list(mats):
    out = np.array([[1.0]], dtype=np.complex128)
    for m in mats:
        out = np.kron(out, m)
    return out


def _m7(beta):
    return _kron_list([_rx(beta)] * 7)


def _m41(beta):
    return _kron_list([_rx(beta)] * 4 + [np.eye(2, dtype=np.complex128)] * 3)


def _bitmap_after_A(bm):
    new = [0] * N
    for j in range(7):
        new[11 + j] = bm[j]
    for j in range(4):
        new[7 + j] = bm[7 + j]
    for j in range(7):
        new[j] = bm[11 + j]
    return new


def _bitmap_after_B(bm):
    # window = free bits 10..4 (single strided AP dim), tiles = bits 3..0
    new = [0] * N
    for j in range(7):
        new[11 + j] = bm[4 + j]
    for j in range(4):
        new[7 + j] = bm[j]
    for j in range(7):
        new[j] = bm[11 + j]
    return new


def _perm_for_bitmap(bm):
    a = np.arange(DIM, dtype=np.int64)
    out = np.zeros(DIM, dtype=np.int64)
    for j in range(N):
        out |= ((a >> j) & 1) << bm[j]
    return out


def _layer_perms():
    """Permutations (orig_idx = perm[cur_idx]) for the state layout at the
    start of each layer (1..LAYERS) plus the final layout (index LAYERS)."""
    perms = []
    bm = list(range(N))
    for _ in range(LAYERS):
        perms.append(_perm_for_bitmap(bm))
        bm = _bitmap_after_B(_bitmap_after_A(bm))
    perms.append(_perm_for_bitmap(bm))
    return perms


_PERMS = _layer_perms()


def _host_prep(batch_betas, adj_matrices):
    """Build per-core input dicts."""
    in_maps = []
    for b in range(BATCH):
        hp = _compute_hp(np.asarray(adj_matrices[b], dtype=np.float64))
        cos_hp = np.cos(hp)
        sin_hp = np.sin(hp)

        def umajor(a):
            # state col permutation c = 128*w + u -> c' = 16*u + w so phase
            # A's stationary windows are stride-16 (LDWEIGHTS then overlaps
            # the previous matmul, like phase B's windows)
            return a.reshape(P, 16, 128).transpose(0, 2, 1).reshape(P, F)

        def chunk_uj(a):
            # per-512-chunk local (u,j)-major order matching the rotation's
            # u-major iteration of phase C's PSUM chunks
            return a.reshape(P, 4, 4, 128).transpose(0, 1, 3, 2).reshape(P, F)

        init_re = umajor(cos_hp[_PERMS[0]].astype(np.float16).reshape(P, F)).copy()
        init_im = umajor((-sin_hp[_PERMS[0]]).astype(np.float16).reshape(P, F)).copy()

        diags = np.empty((2 * (LAYERS - 1) + 1, P, F), dtype=np.float16)
        for t in range(1, LAYERS):
            diags[2 * (t - 1)] = chunk_uj(
                cos_hp[_PERMS[t]].astype(np.float16).reshape(P, F)
            )
            diags[2 * (t - 1) + 1] = chunk_uj(
                sin_hp[_PERMS[t]].astype(np.float16).reshape(P, F)
            )
        diags[-1] = hp[_PERMS[LAYERS]].astype(np.float16).reshape(P, F)

        gates_ab = np.empty((LAYERS, P, 512), dtype=np.float16)
        gates_c = np.empty((LAYERS, P, 384), dtype=np.float16)
        for t in range(LAYERS):
            beta = float(np.asarray(batch_betas[b][t], dtype=np.float64))
            M7 = _m7(beta)
            C7 = M7.real.astype(np.float16)
            D7 = M7.imag.astype(np.float16)
            M41 = _m41(beta)
            C41 = M41.real.astype(np.float16)
            D41 = M41.imag.astype(np.float16)
            gates_ab[t, :, 0:128] = C7
            gates_ab[t, :, 128:256] = D7
            gates_ab[t, :, 256:384] = -D7
            gates_ab[t, :, 384:512] = C7
            gates_c[t, :, 0:128] = C41
            gates_c[t, :, 128:256] = -D41
            gates_c[t, :, 256:384] = D41

        in_maps.append(
            {
                "init_re": init_re,
                "init_im": init_im,
                "diags": diags,
                "gates_ab": gates_ab,
                "gates_c": gates_c,
            }
        )
    return in_maps


# ----------------------------------------------------------------------------
# Bass program
# ----------------------------------------------------------------------------


def _build_program():
    nc = bacc.Bacc("TRN2", target_bir_lowering=False, debug=False)

    d_init_re = nc.dram_tensor("init_re", [P, F], FP16, kind="ExternalInput")
    d_init_im = nc.dram_tensor("init_im", [P, F], FP16, kind="ExternalInput")
    d_diags = nc.dram_tensor(
        "diags", [2 * (LAYERS - 1) + 1, P, F], FP16, kind="ExternalInput"
    )
    d_gates_ab = nc.dram_tensor("gates_ab", [LAYERS, P, 512], FP16, kind="ExternalInput")
    d_gates_c = nc.dram_tensor("gates_c", [LAYERS, P, 384], FP16, kind="ExternalInput")
    d_out = nc.dram_tensor("out", [P, 8], FP32, kind="ExternalOutput")

    n_diag = 2 * (LAYERS - 1) + 1

    def stt(eng, out, in0, in1, op):
        # (in0 * 1.0) op in1 -- plain fp16 tensor_tensor is ~2x slower on DVE
        eng.scalar_tensor_tensor(out, in0, 1.0, in1, ALU.mult, op)

    with tile.TileContext(nc) as tc:
        with (
            tc.tile_pool(name="state", bufs=1) as st_pool,
            tc.tile_pool(name="consts", bufs=1) as c_pool,
            tc.tile_pool(name="rot", bufs=2) as h_pool,
            tc.tile_pool(name="ps_mm", bufs=2, space="PSUM") as ps_pool,
            tc.tile_pool(name="ps_c", bufs=4, space="PSUM") as ps_c,
        ):
            s_re = st_pool.tile([P, F], FP16, tag="s_re")
            s_im = st_pool.tile([P, F], FP16, tag="s_im")
            t1_re = st_pool.tile([P, F], FP16, tag="t1_re")
            t1_im = st_pool.tile([P, F], FP16, tag="t1_im")
            # B output keeps PSUM's interleaved layout (per 128-col window:
            # re|im) so each B drain is one contiguous copy; phase C reads
            # it through a strided moving AP.
            t2 = st_pool.tile([P, 2 * F], FP16, tag="t2")

            diag_t = [
                c_pool.tile([P, F], FP16, tag=f"diag{k}", name=f"diag{k}")
                for k in range(n_diag)
            ]
            gates_ab_t = [
                c_pool.tile([P, 512], FP16, tag=f"gab{t}", name=f"gab{t}")
                for t in range(LAYERS)
            ]
            gates_c_t = [
                c_pool.tile([P, 384], FP16, tag=f"gc{t}", name=f"gc{t}")
                for t in range(LAYERS)
            ]
            partial = c_pool.tile([P, 8], FP32, tag="partial")

            # ---- input DMAs: the first A phase's operands lead each queue.
            # The u-major state layout makes every A window depend on the
            # whole init tensor, so transfer each as one descriptor.
            nc.scalar.dma_start(gates_ab_t[0][:], d_gates_ab.ap()[0])
            nc.scalar.dma_start(s_re[:], d_init_re.ap())
            nc.gpsimd.dma_start(s_im[:], d_init_im.ap())
            nc.gpsimd.dma_start(gates_c_t[0][:], d_gates_c.ap()[0])
            for t in range(1, LAYERS):
                nc.scalar.dma_start(gates_ab_t[t][:], d_gates_ab.ap()[t])
                nc.gpsimd.dma_start(gates_c_t[t][:], d_gates_c.ap()[t])
            nc.gpsimd.dma_start(diag_t[n_diag - 1][:], d_diags.ap()[n_diag - 1])

            def issue_diag_dmas(t):
                if t < LAYERS - 1:
                    nc.scalar.dma_start(diag_t[2 * t][:], d_diags.ap()[2 * t])
                    nc.gpsimd.dma_start(diag_t[2 * t + 1][:], d_diags.ap()[2 * t + 1])

            # strided views: A/B stationary windows / C moving chunks.
            # s_re/s_im live in u-major layout (col = 16*u + w): window w is
            # a stride-16 AP, and the rotation writes chunk k to [:, :, 4k:4k+4].
            s4r = s_re[:].rearrange("p (u w) -> p u w", u=128)
            s4i = s_im[:].rearrange("p (u w) -> p u w", u=128)
            t1r4 = t1_re[:].rearrange("p (w u) -> p w u", w=128)
            t1i4 = t1_im[:].rearrange("p (w u) -> p w u", w=128)
            t2v = t2[:].rearrange("p (u ri j) -> p u ri j", u=16, ri=2)

            for t in range(LAYERS):
                cd7 = gates_ab_t[t][:, 0:256]
                ndc7 = gates_ab_t[t][:, 256:512]
                c41 = gates_c_t[t][:, 0:128]
                nd41 = gates_c_t[t][:, 128:256]
                d41 = gates_c_t[t][:, 256:384]

                # ---- phase A: stationary = state (stride-16 u-major windows)
                for g in range(4):
                    ps = ps_pool.tile([P, 1024], FP32, tag="ps", name=f"psA{t}{g}")
                    for j in range(4):
                        w = 4 * g + j
                        out_sl = ps[:, 256 * j : 256 * (j + 1)]
                        nc.tensor.matmul(out_sl, s4r[:, :, w], cd7, start=True, stop=False)
                        nc.tensor.matmul(out_sl, s4i[:, :, w], ndc7, start=False, stop=True)
                    src = ps[:].rearrange("p (j h) -> p j h", j=4)
                    dst = slice(512 * g, 512 * (g + 1))
                    nc.scalar.copy(t1_re[:, dst], src[:, :, 0:128])
                    nc.scalar.copy(t1_im[:, dst], src[:, :, 128:256])
                    if g == 0:
                        issue_diag_dmas(t)

                # ---- phase B: stationary = t1 stride-16 windows; output
                # drains keep the interleaved layout, split across engines
                # so phase C is not throttled by a single drain chain
                for g in range(4):
                    ps = ps_pool.tile([P, 1024], FP32, tag="ps", name=f"psB{t}{g}")
                    for j in range(4):
                        u = 4 * g + j
                        out_sl = ps[:, 256 * j : 256 * (j + 1)]
                        nc.tensor.matmul(
                            out_sl, t1r4[:, :, u], cd7, start=True, stop=False
                        )
                        nc.tensor.matmul(
                            out_sl, t1i4[:, :, u], ndc7, start=False, stop=True
                        )
                    dst0 = slice(1024 * g, 1024 * g + 512)
                    dst1 = slice(1024 * g + 512, 1024 * (g + 1))
                    nc.scalar.copy(t2[:, dst0], ps[:, 0:512])
                    nc.vector.tensor_copy(t2[:, dst1], ps[:, 512:1024])

                # ---- phase C: standard matmul, gate RX^4 (x) I_8 on partitions.
                # One PSUM bank per chunk so rotation reads never serialize
                # other chunks' matmuls. Rotation reads PSUM directly via
                # DVE STT; gpsimd picks up SBUF-only combines.
                if True:
                    for k in range(4):
                        pcr = ps_c.tile([P, 512], FP32, tag="pc", name=f"pcr{t}{k}")
                        pci = ps_c.tile([P, 512], FP32, tag="pc", name=f"pci{t}{k}")
                        mv_re = t2v[:, 4 * k : 4 * (k + 1), 0, :]
                        mv_im = t2v[:, 4 * k : 4 * (k + 1), 1, :]
                        pre = pcr[:]
                        pim = pci[:]
                        nc.tensor.matmul(pre, c41, mv_re, start=True, stop=False)
                        nc.tensor.matmul(pim, c41, mv_im, start=True, stop=False)
                        nc.tensor.matmul(pre, nd41, mv_im, start=False, stop=True)
                        nc.tensor.matmul(pim, d41, mv_re, start=False, stop=True)

                        ck = slice(512 * k, 512 * (k + 1))
                        if t < LAYERS - 1:
                            # state = psC * exp(-i hp):
                            #   re' = re*c + im*s ; im' = im*c - re*s
                            # Mults iterate the PSUM chunk u-major (diags are
                            # host-permuted to match); combines then write the
                            # u-major state with a short-run strided dst.
                            cs = diag_t[2 * t][:, ck]
                            sn = diag_t[2 * t + 1][:, ck]
                            pre_u = pre.rearrange("p (j u) -> p u j", j=4)
                            pim_u = pim.rearrange("p (j u) -> p u j", j=4)
                            m0 = h_pool.tile([P, 512], FP16, tag="h0", name=f"m0_{t}{k}")
                            m1 = h_pool.tile([P, 512], FP16, tag="h1", name=f"m1_{t}{k}")
                            m2 = h_pool.tile([P, 512], FP16, tag="h2", name=f"m2_{t}{k}")
                            m3 = h_pool.tile([P, 512], FP16, tag="h3", name=f"m3_{t}{k}")
                            dst_re = s4r[:, :, 4 * k : 4 * (k + 1)]
                            dst_im = s4i[:, :, 4 * k : 4 * (k + 1)]

                            def uj(ap):
                                return ap.rearrange("p (u j) -> p u j", j=4)

                            stt(nc.vector, uj(m0[:]), pre_u, uj(cs), ALU.mult)
                            stt(nc.vector, uj(m1[:]), pim_u, uj(sn), ALU.mult)
                            if k == 0:
                                # boundary chunk: lowest-latency path, all DVE
                                stt(nc.vector, dst_re, uj(m0[:]), uj(m1[:]), ALU.add)
                                stt(nc.vector, uj(m3[:]), pim_u, uj(cs), ALU.mult)
                                stt(nc.vector, uj(m2[:]), pre_u, uj(sn), ALU.mult)
                                stt(nc.vector, dst_im, uj(m3[:]), uj(m2[:]), ALU.subtract)
                            else:
                                stt(nc.vector, uj(m3[:]), pim_u, uj(cs), ALU.mult)
                                stt(nc.vector, uj(m2[:]), pre_u, uj(sn), ALU.mult)
                                nc.gpsimd.tensor_tensor(
                                    dst_re, uj(m0[:]), uj(m1[:]), ALU.add
                                )
                                nc.gpsimd.tensor_tensor(
                                    dst_im, uj(m3[:]), uj(m2[:]), ALU.subtract
                                )
                        else:
                            # energy: sum(|state|^2 * hp), separate re/im
                            # partial columns to keep the tail chain short
                            hp_d = diag_t[n_diag - 1][:, ck]
                            sq_re = h_pool.tile([P, 512], FP16, tag="h0", name=f"sqre{k}")
                            sq_im = h_pool.tile([P, 512], FP16, tag="h1", name=f"sqim{k}")
                            dm0 = h_pool.tile([P, 512], FP16, tag="h2", name=f"dmr{k}")
                            dm1 = h_pool.tile([P, 512], FP16, tag="h3", name=f"dmi{k}")
                            nc.scalar.activation(sq_re[:], pre, ACT.Square)
                            nc.scalar.activation(sq_im[:], pim, ACT.Square)
                            nc.vector.scalar_tensor_tensor(
                                dm0[:],
                                sq_re[:],
                                1.0,
                                hp_d,
                                ALU.mult,
                                ALU.mult,
                                accum_out=partial[:, k : k + 1],
                            )
                            nc.vector.scalar_tensor_tensor(
                                dm1[:],
                                sq_im[:],
                                1.0,
                                hp_d,
                                ALU.mult,
                                ALU.mult,
                                accum_out=partial[:, 4 + k : 5 + k],
                            )

            nc.sync.dma_start(d_out.ap(), partial[:])

    nc.compile()
    return nc


_NC_CACHE = {}


def _get_program():
    if "nc" not in _NC_CACHE:
        _NC_CACHE["nc"] = _build_program()
    return _NC_CACHE["nc"]


def kernel(batch_betas, adj_matrices, _trace=False, _tmpdir=None):
    batch_betas = np.asarray(batch_betas, dtype=np.float32)
    adj_matrices = np.asarray(adj_matrices, dtype=np.float32)
    assert batch_betas.shape == (BATCH, LAYERS)
    assert adj_matrices.shape == (BATCH, N, N)

    nc = _get_program()
    in_maps = _host_prep(batch_betas, adj_matrices)
    res = run_bass_kernel_spmd(
        nc,
        in_maps,
        list(range(NCORES)),
        trace=_trace,
        tmpdir=_tmpdir,
    )
    energies = np.array(
        [res.results[b]["out"].sum() / DIM for b in range(BATCH)], dtype=np.float32
    )
    if _trace:
        return energies, res
    return energies


# revision 48
# speedup vs baseline: 1.0537x; 1.0537x over previous
"""TRN2 Bass kernel for nn_DiffQuantumSimulator (QAOA MaxCut, 18 qubits, p=4).

Strategy: data-parallel over batch (8 graphs -> 8 NeuronCores). Per core the
2^18 statevector lives in SBUF as [128 partitions x 2048] (re/im fp16 split).

Each QAOA layer applies exp(-i*hp) (diagonal) and the mixer RX(beta)^(x)18.
The mixer runs in 3 TensorE matmul phases:
  A: 128x128 complex gate RX^(x)7 on the 7 partition bits, fused with a
     partition<->free-bit transpose by using the *state* as the stationary
     operand (out = state_tile^T @ [C|D]).
  B: same trick on the next 7 bits.
  C: standard matmul applying RX^(x)4 (x) I_8 to the remaining 4 bits.
All matmuls run in float16 (full PE rate, half-cost LDWEIGHTS vs fp32r).

The A output (t1) is drained re/im-split for phase B's single-stride
stationary windows (stationary APs allow only one free dim); the B output
(t2) keeps PSUM's interleaved layout (per 128-col window: re|im) so each
B drain is one contiguous copy, and phase C reads it through a strided
moving AP. Phase C uses one PSUM bank per 512-chunk so the rotation's
PSUM reads never serialize other chunks' matmuls.

The inter-layer rotation (multiply by exp(-i*hp)) reads phase C's PSUM
directly via vector-engine scalar_tensor_tensor ops (plain fp16
tensor_tensor is ~2x slower on DVE than STT); gpsimd handles SBUF-only
combine ops, and the boundary chunk takes an all-DVE low-latency path.
Device returns 8 per-partition energy partial columns; host reduces and
scales.
"""

import numpy as np

import concourse.bass as bass
import concourse.mybir as mybir
import concourse.tile as tile
from concourse import bacc
from concourse.bass_utils import run_bass_kernel_spmd

N = 18
DIM = 1 << N
P = 128
F = DIM // P  # 2048
LAYERS = 4
BATCH = 8
NCORES = 8

FP32 = mybir.dt.float32
FP16 = mybir.dt.float16
ALU = mybir.AluOpType
ACT = mybir.ActivationFunctionType

# ----------------------------------------------------------------------------
# Host-side math: hp diagonal, gate matrices, bit-layout permutations
# ----------------------------------------------------------------------------


def _compute_hp(adj):
    W = (np.triu(adj, k=1) > 0.5).astype(np.float64)
    n_edges = W.sum()
    idx = np.arange(DIM)
    shifts = (N - 1 - np.arange(N))[:, None]
    Z = 1.0 - 2.0 * ((idx[None, :] >> shifts) & 1).astype(np.float64)
    T = W @ Z
    cross = np.einsum("ud,ud->d", T, Z)
    return 0.5 * (n_edges - cross)  # [DIM], integer-valued*0.5, exact


def _rx(beta):
    c, s = np.cos(beta), np.sin(beta)
    return np.array([[c, -1j * s], [-1j * s, c]], dtype=np.complex128)


def _kron_list(mats):
    out = np.array([[1.0]], dtype=np.complex128)
    for m in mats:
        out = np.kron(out, m)
    return out


def _m7(beta):
    return _kron_list([_rx(beta)] * 7)


def _m41(beta):
    return _kron_list([_rx(beta)] * 4 + [np.eye(2, dtype=np.complex128)] * 3)


def _bitmap_after_A(bm):
    new = [0] * N
    for j in range(7):
        new[11 + j] = bm[j]
    for j in range(4):
        new[7 + j] = bm[7 + j]
    for j in range(7):
        new[j] = bm[11 + j]
    return new


def _bitmap_after_B(bm):
    # window = free bits 10..4 (single strided AP dim), tiles = bits 3..0
    new = [0] * N
    for j in range(7):
        new[11 + j] = bm[4 + j]
    for j in range(4):
        new[7 + j] = bm[j]
    for j in range(7):
        new[j] = bm[11 + j]
    return new


def _perm_for_bitmap(bm):
    a = np.arange(DIM, dtype=np.int64)
    out = np.zeros(DIM, dtype=np.int64)
    for j in range(N):
        out |= ((a >> j) & 1) << bm[j]
    return out


def _layer_perms():
    """Permutations (orig_idx = perm[cur_idx]) for the state layout at the
    start of each layer (1..LAYERS) plus the final layout (index LAYERS)."""
    perms = []
    bm = list(range(N))
    for _ in range(LAYERS):
        perms.append(_perm_for_bitmap(bm))
        bm = _bitmap_after_B(_bitmap_after_A(bm))
    perms.append(_perm_for_bitmap(bm))
    return perms


_PERMS = _layer_perms()


def _host_prep(batch_betas, adj_matrices):
    """Build per-core input dicts."""
    in_maps = []
    for b in range(BATCH):
        hp = _compute_hp(np.asarray(adj_matrices[b], dtype=np.float64))
        cos_hp = np.cos(hp)
        sin_hp = np.sin(hp)

        init_re = (
            cos_hp[_PERMS[0]].astype(np.float16).reshape(P, 4, 512)
            .transpose(1, 0, 2).copy()
        )
        init_im = (
            (-sin_hp[_PERMS[0]]).astype(np.float16).reshape(P, 4, 512)
            .transpose(1, 0, 2).copy()
        )

        diags = np.empty((2 * (LAYERS - 1) + 1, P, F), dtype=np.float16)
        for t in range(1, LAYERS):
            diags[2 * (t - 1)] = cos_hp[_PERMS[t]].astype(np.float16).reshape(P, F)
            diags[2 * (t - 1) + 1] = sin_hp[_PERMS[t]].astype(np.float16).reshape(P, F)
        diags[-1] = hp[_PERMS[LAYERS]].astype(np.float16).reshape(P, F)

        gates_ab = np.empty((LAYERS, P, 512), dtype=np.float16)
        gates_c = np.empty((LAYERS, P, 384), dtype=np.float16)
        for t in range(LAYERS):
            beta = float(np.asarray(batch_betas[b][t], dtype=np.float64))
            M7 = _m7(beta)
            C7 = M7.real.astype(np.float16)
            D7 = M7.imag.astype(np.float16)
            M41 = _m41(beta)
            C41 = M41.real.astype(np.float16)
            D41 = M41.imag.astype(np.float16)
            gates_ab[t, :, 0:128] = C7
            gates_ab[t, :, 128:256] = D7
            gates_ab[t, :, 256:384] = -D7
            gates_ab[t, :, 384:512] = C7
            gates_c[t, :, 0:128] = C41
            gates_c[t, :, 128:256] = -D41
            gates_c[t, :, 256:384] = D41

        in_maps.append(
            {
                "init_re": init_re,
                "init_im": init_im,
                "diags": diags,
                "gates_ab": gates_ab,
                "gates_c": gates_c,
            }
        )
    return in_maps


# ----------------------------------------------------------------------------
# Bass program
# ----------------------------------------------------------------------------


def _build_program():
    nc = bacc.Bacc("TRN2", target_bir_lowering=False, debug=False)

    d_init_re = nc.dram_tensor("init_re", [4, P, 512], FP16, kind="ExternalInput")
    d_init_im = nc.dram_tensor("init_im", [4, P, 512], FP16, kind="ExternalInput")
    d_diags = nc.dram_tensor(
        "diags", [2 * (LAYERS - 1) + 1, P, F], FP16, kind="ExternalInput"
    )
    d_gates_ab = nc.dram_tensor("gates_ab", [LAYERS, P, 512], FP16, kind="ExternalInput")
    d_gates_c = nc.dram_tensor("gates_c", [LAYERS, P, 384], FP16, kind="ExternalInput")
    d_out = nc.dram_tensor("out", [P, 8], FP32, kind="ExternalOutput")

    n_diag = 2 * (LAYERS - 1) + 1

    def stt(eng, out, in0, in1, op):
        # (in0 * 1.0) op in1 -- plain fp16 tensor_tensor is ~2x slower on DVE
        eng.scalar_tensor_tensor(out, in0, 1.0, in1, ALU.mult, op)

    with tile.TileContext(nc) as tc:
        with (
            tc.tile_pool(name="state", bufs=1) as st_pool,
            tc.tile_pool(name="consts", bufs=1) as c_pool,
            tc.tile_pool(name="rot", bufs=2) as h_pool,
            tc.tile_pool(name="ps_mm", bufs=2, space="PSUM") as ps_pool,
            tc.tile_pool(name="ps_c", bufs=4, space="PSUM") as ps_c,
        ):
            s_re = st_pool.tile([P, F], FP16, tag="s_re")
            s_im = st_pool.tile([P, F], FP16, tag="s_im")
            t1_re = st_pool.tile([P, F], FP16, tag="t1_re")
            t1_im = st_pool.tile([P, F], FP16, tag="t1_im")
            # B output keeps PSUM's interleaved layout (per 128-col window:
            # re|im) so each B drain is one contiguous copy; phase C reads
            # it through a strided moving AP.
            t2 = st_pool.tile([P, 2 * F], FP16, tag="t2")

            diag_t = [
                c_pool.tile([P, F], FP16, tag=f"diag{k}", name=f"diag{k}")
                for k in range(n_diag)
            ]
            gates_ab_t = [
                c_pool.tile([P, 512], FP16, tag=f"gab{t}", name=f"gab{t}")
                for t in range(LAYERS)
            ]
            gates_c_t = [
                c_pool.tile([P, 384], FP16, tag=f"gc{t}", name=f"gc{t}")
                for t in range(LAYERS)
            ]
            partial = c_pool.tile([P, 8], FP32, tag="partial")

            # ---- input DMAs: the first A-group's operands lead each queue
            # so the first matmul can start after ~3 descriptors, not 9
            nc.scalar.dma_start(gates_ab_t[0][:], d_gates_ab.ap()[0])
            nc.scalar.dma_start(s_re[:, 0:512], d_init_re.ap()[0])
            nc.gpsimd.dma_start(s_im[:, 0:512], d_init_im.ap()[0])
            for c in range(1, 4):
                q = nc.scalar if c % 2 == 1 else nc.gpsimd
                q.dma_start(s_re[:, 512 * c : 512 * (c + 1)], d_init_re.ap()[c])
                q2 = nc.gpsimd if c % 2 == 1 else nc.scalar
                q2.dma_start(s_im[:, 512 * c : 512 * (c + 1)], d_init_im.ap()[c])
            nc.gpsimd.dma_start(gates_c_t[0][:], d_gates_c.ap()[0])
            for t in range(1, LAYERS):
                nc.scalar.dma_start(gates_ab_t[t][:], d_gates_ab.ap()[t])
                nc.gpsimd.dma_start(gates_c_t[t][:], d_gates_c.ap()[t])
            nc.gpsimd.dma_start(diag_t[n_diag - 1][:], d_diags.ap()[n_diag - 1])

            def issue_diag_dmas(t):
                if t < LAYERS - 1:
                    nc.scalar.dma_start(diag_t[2 * t][:], d_diags.ap()[2 * t])
                    nc.gpsimd.dma_start(diag_t[2 * t + 1][:], d_diags.ap()[2 * t + 1])

            # strided views: B stationary windows / C moving chunks
            t1r4 = t1_re[:].rearrange("p (w u) -> p w u", w=128)
            t1i4 = t1_im[:].rearrange("p (w u) -> p w u", w=128)
            t2v = t2[:].rearrange("p (u ri j) -> p u ri j", u=16, ri=2)

            for t in range(LAYERS):
                cd7 = gates_ab_t[t][:, 0:256]
                ndc7 = gates_ab_t[t][:, 256:512]
                c41 = gates_c_t[t][:, 0:128]
                nd41 = gates_c_t[t][:, 128:256]
                d41 = gates_c_t[t][:, 256:384]

                # ---- phase A: stationary = state (contiguous 128-col windows)
                for g in range(4):
                    ps = ps_pool.tile([P, 1024], FP32, tag="ps", name=f"psA{t}{g}")
                    for j in range(4):
                        w = 4 * g + j
                        sl = slice(128 * w, 128 * (w + 1))
                        out_sl = ps[:, 256 * j : 256 * (j + 1)]
                        nc.tensor.matmul(out_sl, s_re[:, sl], cd7, start=True, stop=False)
                        nc.tensor.matmul(out_sl, s_im[:, sl], ndc7, start=False, stop=True)
                    src = ps[:].rearrange("p (j h) -> p j h", j=4)
                    dst = slice(512 * g, 512 * (g + 1))
                    nc.scalar.copy(t1_re[:, dst], src[:, :, 0:128])
                    nc.scalar.copy(t1_im[:, dst], src[:, :, 128:256])
                    if g == 0:
                        issue_diag_dmas(t)

                # ---- phase B: stationary = t1 stride-16 windows; output
                # drains keep the interleaved layout, split across engines
                # so phase C is not throttled by a single drain chain
                for g in range(4):
                    ps = ps_pool.tile([P, 1024], FP32, tag="ps", name=f"psB{t}{g}")
                    for j in range(4):
                        u = 4 * g + j
                        out_sl = ps[:, 256 * j : 256 * (j + 1)]
                        nc.tensor.matmul(
                            out_sl, t1r4[:, :, u], cd7, start=True, stop=False
                        )
                        nc.tensor.matmul(
                            out_sl, t1i4[:, :, u], ndc7, start=False, stop=True
                        )
                    dst0 = slice(1024 * g, 1024 * g + 512)
                    dst1 = slice(1024 * g + 512, 1024 * (g + 1))
                    nc.scalar.copy(t2[:, dst0], ps[:, 0:512])
                    nc.vector.tensor_copy(t2[:, dst1], ps[:, 512:1024])

                # ---- phase C: standard matmul, gate RX^4 (x) I_8 on partitions.
                # One PSUM bank per chunk so rotation reads never serialize
                # other chunks' matmuls. Rotation reads PSUM directly via
                # DVE STT; gpsimd picks up SBUF-only combines.
                if True:
                    for k in range(4):
                        pcr = ps_c.tile([P, 512], FP32, tag="pc", name=f"pcr{t}{k}")
                        pci = ps_c.tile([P, 512], FP32, tag="pc", name=f"pci{t}{k}")
                        mv_re = t2v[:, 4 * k : 4 * (k + 1), 0, :]
                        mv_im = t2v[:, 4 * k : 4 * (k + 1), 1, :]
                        pre = pcr[:]
                        pim = pci[:]
                        nc.tensor.matmul(pre, c41, mv_re, start=True, stop=False)
                        nc.tensor.matmul(pim, c41, mv_im, start=True, stop=False)
                        nc.tensor.matmul(pre, nd41, mv_im, start=False, stop=True)
                        nc.tensor.matmul(pim, d41, mv_re, start=False, stop=True)

                        ck = slice(512 * k, 512 * (k + 1))
                        if t < LAYERS - 1:
                            # state = psC * exp(-i hp):
                            #   re' = re*c + im*s ; im' = im*c - re*s
                            cs = diag_t[2 * t][:, ck]
                            sn = diag_t[2 * t + 1][:, ck]
                            m0 = h_pool.tile([P, 512], FP16, tag="h0", name=f"m0_{t}{k}")
                            m1 = h_pool.tile([P, 512], FP16, tag="h1", name=f"m1_{t}{k}")
                            m2 = h_pool.tile([P, 512], FP16, tag="h2", name=f"m2_{t}{k}")
                            m3 = h_pool.tile([P, 512], FP16, tag="h3", name=f"m3_{t}{k}")
                            stt(nc.vector, m0[:], pre, cs, ALU.mult)
                            stt(nc.vector, m1[:], pim, sn, ALU.mult)
                            if k == 0:
                                # boundary chunk: lowest-latency path, all DVE
                                stt(nc.vector, s_re[:, ck], m0[:], m1[:], ALU.add)
                                stt(nc.vector, m3[:], pim, cs, ALU.mult)
                                stt(nc.vector, m2[:], pre, sn, ALU.mult)
                                stt(nc.vector, s_im[:, ck], m3[:], m2[:], ALU.subtract)
                            else:
                                stt(nc.vector, m3[:], pim, cs, ALU.mult)
                                stt(nc.vector, m2[:], pre, sn, ALU.mult)
                                nc.gpsimd.tensor_tensor(
                                    s_re[:, ck], m0[:], m1[:], ALU.add
                                )
                                nc.gpsimd.tensor_tensor(
                                    s_im[:, ck], m3[:], m2[:], ALU.subtract
                                )
                        else:
                            # energy: sum(|state|^2 * hp), separate re/im
                            # partial columns to keep the tail chain short
                            hp_d = diag_t[n_diag - 1][:, ck]
                            sq_re = h_pool.tile([P, 512], FP16, tag="h0", name=f"sqre{k}")
                            sq_im = h_pool.tile([P, 512], FP16, tag="h1", name=f"sqim{k}")
                            dm0 = h_pool.tile([P, 512], FP16, tag="h2", name=f"dmr{k}")
                            dm1 = h_pool.tile([P, 512], FP16, tag="h3", name=f"dmi{k}")
                            nc.scalar.activation(sq_re[:], pre, ACT.Square)
                            nc.scalar.activation(sq_im[:], pim, ACT.Square)
                            nc.vector.scalar_tensor_tensor(
                                dm0[:],
                                sq_re[:],
                                1.0,
                                hp_d,
                                ALU.mult,
                                ALU.mult,
                                accum_out=partial[:, k : k + 1],
                            )
                            nc.vector.scalar_tensor_tensor(
                                dm1[:],
                                sq_im[:],
                                1.0,
                                hp_d,
                                ALU.mult,
                                ALU.mult,
                                accum_out=partial[:, 4 + k : 5 + k],
                            )

            nc.sync.dma_start(d_out.ap(), partial[:])

    nc.compile()
    return nc


_NC_CACHE = {}


def _get_program():
    if "nc" not in _NC_CACHE:
        _NC_CACHE["nc"] = _build_program()
    return _NC_CACHE["nc"]


def kernel(batch_betas, adj_matrices, _trace=False, _tmpdir=None):
    batch_betas = np.asarray(batch_betas, dtype=np.float32)
    adj_matrices = np.asarray(adj_matrices, dtype=np.float32)
    assert batch_betas.shape == (BATCH, LAYERS)
    assert adj_matrices.shape == (BATCH, N, N)

    nc = _get_program()
    in_maps = _host_prep(batch_betas, adj_matrices)
    res = run_bass_kernel_spmd(
        nc,
        in_maps,
        list(range(NCORES)),
        trace=_trace,
        tmpdir=_tmpdir,
    )
    energies = np.array(
        [res.results[b]["out"].sum() / DIM for b in range(BATCH)], dtype=np.float32
    )
    if _trace:
        return energies, res
    return energies


# revision 49
# speedup vs baseline: 1.0887x; 1.0332x over previous
"""TRN2 Bass kernel for nn_DiffQuantumSimulator (QAOA MaxCut, 18 qubits, p=4).

Strategy: data-parallel over batch (8 graphs -> 8 NeuronCores). Per core the
2^18 statevector lives in SBUF as [128 partitions x 2048] (re/im fp16 split).

Each QAOA layer applies exp(-i*hp) (diagonal) and the mixer RX(beta)^(x)18.
The mixer runs in 3 TensorE matmul phases:
  A: 128x128 complex gate RX^(x)7 on the 7 partition bits, fused with a
     partition<->free-bit transpose by using the *state* as the stationary
     operand (out = state_tile^T @ [C|D]).
  B: same trick on the next 7 bits.
  C: standard matmul applying RX^(x)4 (x) I_8 to the remaining 4 bits.
All matmuls run in float16 (full PE rate, half-cost LDWEIGHTS vs fp32r).

The A output (t1) is drained re/im-split for phase B's single-stride
stationary windows (stationary APs allow only one free dim); the B output
(t2) keeps PSUM's interleaved layout (per 128-col window: re|im) so each
B drain is one contiguous copy, and phase C reads it through a strided
moving AP. Phase C uses one PSUM bank per 512-chunk so the rotation's
PSUM reads never serialize other chunks' matmuls.

The inter-layer rotation (multiply by exp(-i*hp)) reads phase C's PSUM
directly via vector-engine scalar_tensor_tensor ops (plain fp16
tensor_tensor is ~2x slower on DVE than STT); gpsimd handles SBUF-only
combine ops, and the boundary chunk takes an all-DVE low-latency path.
Device returns 8 per-partition energy partial columns; host reduces and
scales.
"""

import numpy as np

import concourse.bass as bass
import concourse.mybir as mybir
import concourse.tile as tile
from concourse import bacc
from concourse.bass_utils import run_bass_kernel_spmd

N = 18
DIM = 1 << N
P = 128
F = DIM // P  # 2048
LAYERS = 4
BATCH = 8
NCORES = 8

FP32 = mybir.dt.float32
FP16 = mybir.dt.float16
ALU = mybir.AluOpType
ACT = mybir.ActivationFunctionType

# ----------------------------------------------------------------------------
# Host-side math: hp diagonal, gate matrices, bit-layout permutations
# ----------------------------------------------------------------------------


def _compute_hp(adj):
    W = (np.triu(adj, k=1) > 0.5).astype(np.float64)
    n_edges = W.sum()
    idx = np.arange(DIM)
    shifts = (N - 1 - np.arange(N))[:, None]
    Z = 1.0 - 2.0 * ((idx[None, :] >> shifts) & 1).astype(np.float64)
    T = W @ Z
    cross = np.einsum("ud,ud->d", T, Z)
    return 0.5 * (n_edges - cross)  # [DIM], integer-valued*0.5, exact


def _rx(beta):
    c, s = np.cos(beta), np.sin(beta)
    return np.array([[c, -1j * s], [-1j * s, c]], dtype=np.complex128)


def _kron_list(mats):
    out = np.array([[1.0]], dtype=np.complex128)
    for m in mats:
        out = np.kron(out, m)
    return out


def _m7(beta):
    return _kron_list([_rx(beta)] * 7)


def _m41(beta):
    return _kron_list([_rx(beta)] * 4 + [np.eye(2, dtype=np.complex128)] * 3)


def _bitmap_after_A(bm):
    new = [0] * N
    for j in range(7):
        new[11 + j] = bm[j]
    for j in range(4):
        new[7 + j] = bm[7 + j]
    for j in range(7):
        new[j] = bm[11 + j]
    return new


def _bitmap_after_B(bm):
    # window = free bits 10..4 (single strided AP dim), tiles = bits 3..0
    new = [0] * N
    for j in range(7):
        new[11 + j] = bm[4 + j]
    for j in range(4):
        new[7 + j] = bm[j]
    for j in range(7):
        new[j] = bm[11 + j]
    return new


def _perm_for_bitmap(bm):
    a = np.arange(DIM, dtype=np.int64)
    out = np.zeros(DIM, dtype=np.int64)
    for j in range(N):
        out |= ((a >> j) & 1) << bm[j]
    return out


def _layer_perms():
    """Permutations (orig_idx = perm[cur_idx]) for the state layout at the
    start of each layer (1..LAYERS) plus the final layout (index LAYERS)."""
    perms = []
    bm = list(range(N))
    for _ in range(LAYERS):
        perms.append(_perm_for_bitmap(bm))
        bm = _bitmap_after_B(_bitmap_after_A(bm))
    perms.append(_perm_for_bitmap(bm))
    return perms


_PERMS = _layer_perms()


def _host_prep(batch_betas, adj_matrices):
    """Build per-core input dicts."""
    in_maps = []
    for b in range(BATCH):
        hp = _compute_hp(np.asarray(adj_matrices[b], dtype=np.float64))
        cos_hp = np.cos(hp)
        sin_hp = np.sin(hp)

        init_re = (
            cos_hp[_PERMS[0]].astype(np.float16).reshape(P, 4, 512)
            .transpose(1, 0, 2).copy()
        )
        init_im = (
            (-sin_hp[_PERMS[0]]).astype(np.float16).reshape(P, 4, 512)
            .transpose(1, 0, 2).copy()
        )

        diags = np.empty((2 * (LAYERS - 1) + 1, P, F), dtype=np.float16)
        for t in range(1, LAYERS):
            diags[2 * (t - 1)] = cos_hp[_PERMS[t]].astype(np.float16).reshape(P, F)
            diags[2 * (t - 1) + 1] = sin_hp[_PERMS[t]].astype(np.float16).reshape(P, F)
        diags[-1] = hp[_PERMS[LAYERS]].astype(np.float16).reshape(P, F)

        gates_ab = np.empty((LAYERS, P, 512), dtype=np.float16)
        gates_c = np.empty((LAYERS, P, 384), dtype=np.float16)
        for t in range(LAYERS):
            beta = float(np.asarray(batch_betas[b][t], dtype=np.float64))
            M7 = _m7(beta)
            C7 = M7.real.astype(np.float16)
            D7 = M7.imag.astype(np.float16)
            M41 = _m41(beta)
            C41 = M41.real.astype(np.float16)
            D41 = M41.imag.astype(np.float16)
            gates_ab[t, :, 0:128] = C7
            gates_ab[t, :, 128:256] = D7
            gates_ab[t, :, 256:384] = -D7
            gates_ab[t, :, 384:512] = C7
            gates_c[t, :, 0:128] = C41
            gates_c[t, :, 128:256] = -D41
            gates_c[t, :, 256:384] = D41

        in_maps.append(
            {
                "init_re": init_re,
                "init_im": init_im,
                "diags": diags,
                "gates_ab": gates_ab,
                "gates_c": gates_c,
            }
        )
    return in_maps


# ----------------------------------------------------------------------------
# Bass program
# ----------------------------------------------------------------------------


def _build_program():
    nc = bacc.Bacc("TRN2", target_bir_lowering=False, debug=False)

    d_init_re = nc.dram_tensor("init_re", [4, P, 512], FP16, kind="ExternalInput")
    d_init_im = nc.dram_tensor("init_im", [4, P, 512], FP16, kind="ExternalInput")
    d_diags = nc.dram_tensor(
        "diags", [2 * (LAYERS - 1) + 1, P, F], FP16, kind="ExternalInput"
    )
    d_gates_ab = nc.dram_tensor("gates_ab", [LAYERS, P, 512], FP16, kind="ExternalInput")
    d_gates_c = nc.dram_tensor("gates_c", [LAYERS, P, 384], FP16, kind="ExternalInput")
    d_out = nc.dram_tensor("out", [P, 8], FP32, kind="ExternalOutput")

    n_diag = 2 * (LAYERS - 1) + 1

    def stt(eng, out, in0, in1, op):
        # (in0 * 1.0) op in1 -- plain fp16 tensor_tensor is ~2x slower on DVE
        eng.scalar_tensor_tensor(out, in0, 1.0, in1, ALU.mult, op)

    with tile.TileContext(nc) as tc:
        with (
            tc.tile_pool(name="state", bufs=1) as st_pool,
            tc.tile_pool(name="consts", bufs=1) as c_pool,
            tc.tile_pool(name="rot", bufs=4) as h_pool,
            tc.tile_pool(name="ps_mm", bufs=2, space="PSUM") as ps_pool,
            tc.tile_pool(name="ps_c", bufs=4, space="PSUM") as ps_c,
        ):
            s_re = st_pool.tile([P, F], FP16, tag="s_re")
            s_im = st_pool.tile([P, F], FP16, tag="s_im")
            t1_re = st_pool.tile([P, F], FP16, tag="t1_re")
            t1_im = st_pool.tile([P, F], FP16, tag="t1_im")
            # B output keeps PSUM's interleaved layout (per 128-col window:
            # re|im) so each B drain is one contiguous copy; phase C reads
            # it through a strided moving AP.
            t2 = st_pool.tile([P, 2 * F], FP16, tag="t2")

            diag_t = [
                c_pool.tile([P, F], FP16, tag=f"diag{k}", name=f"diag{k}")
                for k in range(n_diag)
            ]
            gates_ab_t = [
                c_pool.tile([P, 512], FP16, tag=f"gab{t}", name=f"gab{t}")
                for t in range(LAYERS)
            ]
            gates_c_t = [
                c_pool.tile([P, 384], FP16, tag=f"gc{t}", name=f"gc{t}")
                for t in range(LAYERS)
            ]
            partial = c_pool.tile([P, 8], FP32, tag="partial")

            # ---- input DMAs: the first A-group's operands lead each queue
            # so the first matmul can start after ~3 descriptors, not 9
            nc.scalar.dma_start(gates_ab_t[0][:], d_gates_ab.ap()[0])
            nc.scalar.dma_start(s_re[:, 0:512], d_init_re.ap()[0])
            nc.gpsimd.dma_start(s_im[:, 0:512], d_init_im.ap()[0])
            for c in range(1, 4):
                q = nc.scalar if c % 2 == 1 else nc.gpsimd
                q.dma_start(s_re[:, 512 * c : 512 * (c + 1)], d_init_re.ap()[c])
                q2 = nc.gpsimd if c % 2 == 1 else nc.scalar
                q2.dma_start(s_im[:, 512 * c : 512 * (c + 1)], d_init_im.ap()[c])
            nc.gpsimd.dma_start(gates_c_t[0][:], d_gates_c.ap()[0])
            for t in range(1, LAYERS):
                nc.scalar.dma_start(gates_ab_t[t][:], d_gates_ab.ap()[t])
                nc.gpsimd.dma_start(gates_c_t[t][:], d_gates_c.ap()[t])
            nc.gpsimd.dma_start(diag_t[n_diag - 1][:], d_diags.ap()[n_diag - 1])

            def issue_diag_dmas(t):
                if t < LAYERS - 1:
                    nc.scalar.dma_start(diag_t[2 * t][:], d_diags.ap()[2 * t])
                    nc.gpsimd.dma_start(diag_t[2 * t + 1][:], d_diags.ap()[2 * t + 1])

            # strided views: B stationary windows / C moving chunks
            t1r4 = t1_re[:].rearrange("p (w u) -> p w u", w=128)
            t1i4 = t1_im[:].rearrange("p (w u) -> p w u", w=128)
            t2v = t2[:].rearrange("p (u ri j) -> p u ri j", u=16, ri=2)

            for t in range(LAYERS):
                cd7 = gates_ab_t[t][:, 0:256]
                ndc7 = gates_ab_t[t][:, 256:512]
                c41 = gates_c_t[t][:, 0:128]
                nd41 = gates_c_t[t][:, 128:256]
                d41 = gates_c_t[t][:, 256:384]

                # ---- phase A: stationary = state (contiguous 128-col windows)
                for g in range(4):
                    ps = ps_pool.tile([P, 1024], FP32, tag="ps", name=f"psA{t}{g}")
                    for j in range(4):
                        w = 4 * g + j
                        sl = slice(128 * w, 128 * (w + 1))
                        out_sl = ps[:, 256 * j : 256 * (j + 1)]
                        nc.tensor.matmul(out_sl, s_re[:, sl], cd7, start=True, stop=False)
                        nc.tensor.matmul(out_sl, s_im[:, sl], ndc7, start=False, stop=True)
                    src = ps[:].rearrange("p (j h) -> p j h", j=4)
                    dst = slice(512 * g, 512 * (g + 1))
                    nc.scalar.copy(t1_re[:, dst], src[:, :, 0:128])
                    nc.scalar.copy(t1_im[:, dst], src[:, :, 128:256])
                    if g == 0:
                        issue_diag_dmas(t)

                # ---- phase B: stationary = t1 stride-16 windows; output
                # drains keep the interleaved layout, split across engines
                # so phase C is not throttled by a single drain chain
                for g in range(4):
                    ps = ps_pool.tile([P, 1024], FP32, tag="ps", name=f"psB{t}{g}")
                    for j in range(4):
                        u = 4 * g + j
                        out_sl = ps[:, 256 * j : 256 * (j + 1)]
                        nc.tensor.matmul(
                            out_sl, t1r4[:, :, u], cd7, start=True, stop=False
                        )
                        nc.tensor.matmul(
                            out_sl, t1i4[:, :, u], ndc7, start=False, stop=True
                        )
                    dst0 = slice(1024 * g, 1024 * g + 512)
                    dst1 = slice(1024 * g + 512, 1024 * (g + 1))
                    nc.scalar.copy(t2[:, dst0], ps[:, 0:512])
                    nc.vector.tensor_copy(t2[:, dst1], ps[:, 512:1024])

                # ---- phase C: standard matmul, gate RX^4 (x) I_8 on partitions.
                # One PSUM bank per chunk so rotation reads never serialize
                # other chunks' matmuls. Rotation reads PSUM directly via
                # DVE STT; gpsimd picks up SBUF-only combines.
                if True:
                    for k in range(4):
                        pcr = ps_c.tile([P, 512], FP32, tag="pc", name=f"pcr{t}{k}")
                        pci = ps_c.tile([P, 512], FP32, tag="pc", name=f"pci{t}{k}")
                        mv_re = t2v[:, 4 * k : 4 * (k + 1), 0, :]
                        mv_im = t2v[:, 4 * k : 4 * (k + 1), 1, :]
                        pre = pcr[:]
                        pim = pci[:]
                        nc.tensor.matmul(pre, c41, mv_re, start=True, stop=False)
                        nc.tensor.matmul(pim, c41, mv_im, start=True, stop=False)
                        nc.tensor.matmul(pre, nd41, mv_im, start=False, stop=True)
                        nc.tensor.matmul(pim, d41, mv_re, start=False, stop=True)

                        ck = slice(512 * k, 512 * (k + 1))
                        if t < LAYERS - 1:
                            # state = psC * exp(-i hp):
                            #   re' = re*c + im*s ; im' = im*c - re*s
                            cs = diag_t[2 * t][:, ck]
                            sn = diag_t[2 * t + 1][:, ck]
                            m0 = h_pool.tile([P, 512], FP16, tag="h0", name=f"m0_{t}{k}")
                            m1 = h_pool.tile([P, 512], FP16, tag="h1", name=f"m1_{t}{k}")
                            m2 = h_pool.tile([P, 512], FP16, tag="h2", name=f"m2_{t}{k}")
                            m3 = h_pool.tile([P, 512], FP16, tag="h3", name=f"m3_{t}{k}")
                            stt(nc.vector, m0[:], pre, cs, ALU.mult)
                            stt(nc.vector, m1[:], pim, sn, ALU.mult)
                            if k == 0:
                                # boundary chunk: lowest-latency path, all DVE
                                stt(nc.vector, s_re[:, ck], m0[:], m1[:], ALU.add)
                                stt(nc.vector, m3[:], pim, cs, ALU.mult)
                                stt(nc.vector, m2[:], pre, sn, ALU.mult)
                                stt(nc.vector, s_im[:, ck], m3[:], m2[:], ALU.subtract)
                            else:
                                stt(nc.vector, m3[:], pim, cs, ALU.mult)
                                stt(nc.vector, m2[:], pre, sn, ALU.mult)
                                nc.gpsimd.tensor_tensor(
                                    s_re[:, ck], m0[:], m1[:], ALU.add
                                )
                                nc.gpsimd.tensor_tensor(
                                    s_im[:, ck], m3[:], m2[:], ALU.subtract
                                )
                        else:
                            # energy: sum(|state|^2 * hp), separate re/im
                            # partial columns to keep the tail chain short
                            hp_d = diag_t[n_diag - 1][:, ck]
                            sq_re = h_pool.tile([P, 512], FP16, tag="h0", name=f"sqre{k}")
                            sq_im = h_pool.tile([P, 512], FP16, tag="h1", name=f"sqim{k}")
                            dm0 = h_pool.tile([P, 512], FP16, tag="h2", name=f"dmr{k}")
                            dm1 = h_pool.tile([P, 512], FP16, tag="h3", name=f"dmi{k}")
                            nc.scalar.activation(sq_re[:], pre, ACT.Square)
                            nc.scalar.activation(sq_im[:], pim, ACT.Square)
                            nc.vector.scalar_tensor_tensor(
                                dm0[:],
                                sq_re[:],
                                1.0,
                                hp_d,
                                ALU.mult,
                                ALU.mult,
                                accum_out=partial[:, k : k + 1],
                            )
                            nc.vector.scalar_tensor_tensor(
                                dm1[:],
                                sq_im[:],
                                1.0,
                                hp_d,
                                ALU.mult,
                                ALU.mult,
                                accum_out=partial[:, 4 + k : 5 + k],
                            )

            nc.sync.dma_start(d_out.ap(), partial[:])

    nc.compile()
    return nc


_NC_CACHE = {}


def _get_program():
    if "nc" not in _NC_CACHE:
        _NC_CACHE["nc"] = _build_program()
    return _NC_CACHE["nc"]


def kernel(batch_betas, adj_matrices, _trace=False, _tmpdir=None):
    batch_betas = np.asarray(batch_betas, dtype=np.float32)
    adj_matrices = np.asarray(adj_matrices, dtype=np.float32)
    assert batch_betas.shape == (BATCH, LAYERS)
    assert adj_matrices.shape == (BATCH, N, N)

    nc = _get_program()
    in_maps = _host_prep(batch_betas, adj_matrices)
    res = run_bass_kernel_spmd(
        nc,
        in_maps,
        list(range(NCORES)),
        trace=_trace,
        tmpdir=_tmpdir,
    )
    energies = np.array(
        [res.results[b]["out"].sum() / DIM for b in range(BATCH)], dtype=np.float32
    )
    if _trace:
        return energies, res
    return energies
